# revision 6
# baseline (speedup 1.0000x reference)
"""Differentiable voxel rasterizer — Trainium2 Bass kernel (8 NeuronCores).

Contract: kernel(**inputs) takes FULL inputs (positions [512,3], sizes [512],
densities [512], colors [512,3], camera_matrix [4,4], intrinsics [3,3]) and
returns the FULL output tuple (rgb [256,256,3], depth [256,256],
alpha [256,256]) matching reference.reference().

Strategy (sharding = image plane): each of 8 cores renders a 32-row band.
Within a band, 64 tiles of 8x16 px (partition dim = 128 pixels). Host
projects voxels, culls per tile (exact: outside the circular footprint the
reference weight is identically 0), sorts back-to-front, and packs per-tile
tables. Device computes, per tile:
    d2|t       via one PE matmul (rank-4 expansion, tile-centered coords)
    dist       = sqrt(relu(d2))            (ACT)
    pen        = relu(BIG*t)               (DVE, t = d2-hs^2 mask penalty)
    E          = exp(-0.5*(dist+pen))      (ACT; gpsimd add)
    W          = E*va                      (gpsimd)
    P          = prefix prod of (1-W)      (DVE tensor_tensor_scan)
    blend_v    = P_{v-1}-P_v (telescoping) -> omb = 1-blend via one stt
    R          = suffix prod of omb        (DVE reversed scan)
    BQ_v       = R_{v+1}-R_v (telescoping) = blend_v * prod_{u>v} omb_u
    rgb/depth  = matmul(BQ^T, [c | d-FAR]) (PE transpose + PE matmul)
    alpha      = 1 - P_total
Everything is NaN-free by construction (device NaNs crash the NRT).
"""
import os
import sys

for _p in ("/opt/trn_rl_repo", os.path.expanduser("~/.axon_site/_ro/trn_rl_repo")):
    if os.path.isdir(_p) and _p not in sys.path:
        sys.path.insert(0, _p)

import numpy as np

H, W_IMG = 256, 256
NEAR, FAR = 0.1, 100.0
NVOX = 512
NCORES = 8
BAND = H // NCORES          # 32 rows per core
TR, TC = 4, 16              # tile grid per band
R_, C_ = 8, 16              # tile pixel shape (R_*C_ = 128 partitions)
T = TR * TC                 # 64 tiles per core
P = 128
BIG = 1e8

_nc_cache: dict = {}


def _build(V):
    """Build + compile the per-core module for V voxels per tile."""
    import concourse.bacc as bacc
    import concourse.bass as bass
    import concourse.mybir as mybir
    from concourse.tile import TileContext

    F32 = mybir.dt.float32
    Alu = mybir.AluOpType
    Act = mybir.ActivationFunctionType
    VC = (V + P - 1) // P          # voxel chunks for transpose/reduction
    assert V <= 512

    nc = bacc.Bacc("TRN2", target_bir_lowering=False, debug=False)
    pxf_d = nc.dram_tensor("pxf", [T, 4, P], F32, kind="ExternalInput")
    rhs_d = nc.dram_tensor("rhs8", [T, 4, 2 * V], F32, kind="ExternalInput")
    vab_d = nc.dram_tensor("vab", [T, V], F32, kind="ExternalInput")
    vals_d = nc.dram_tensor("vals", [T, P, VC, 4], F32, kind="ExternalInput")
    id_d = nc.dram_tensor("ident", [P, P], F32, kind="ExternalInput")
    # outputs in device-natural layout [pixel-in-tile, tile(,ch)];
    # host unscatters to image layout during gather
    rgb_d = nc.dram_tensor("rgb", [P, T, 3], F32, kind="ExternalOutput")
    dep_d = nc.dram_tensor("dep", [P, T], F32, kind="ExternalOutput")
    alp_d = nc.dram_tensor("alp", [P, T], F32, kind="ExternalOutput")

    with TileContext(nc) as tc:
        with tc.tile_pool(name="const", bufs=1) as cpool, \
             tc.tile_pool(name="work", bufs=3) as pool, \
             tc.tile_pool(name="pwork", bufs=2, space="PSUM") as ppool, \
             tc.tile_pool(name="pout", bufs=1, space="PSUM") as opool:
            ident = cpool.tile([P, P], F32)
            zeros = cpool.tile([P, V], F32)
            astage = cpool.tile([P, T], F32)
            nc.sync.dma_start(out=ident[:], in_=id_d.ap())
            nc.vector.memset(zeros[:], 0)
            outp = opool.tile([P, 4 * T], F32)

            for t in range(T):
                pxf = pool.tile([4, P], F32)
                rhs8 = pool.tile([4, 2 * V], F32)
                vab = pool.tile([P, V], F32)
                vals = pool.tile([P, VC, 4], F32)
                nc.sync.dma_start(out=pxf[:], in_=pxf_d.ap()[t])
                nc.sync.dma_start(out=rhs8[:], in_=rhs_d.ap()[t])
                nc.sync.dma_start(out=vals[:], in_=vals_d.ap()[t])
                row = vab_d.ap()[t:t + 1, :]
                nc.sync.dma_start(
                    out=vab[:],
                    in_=bass.AP(row.tensor, row.offset, [[0, P], [1, V]]))

                d2t = ppool.tile([P, 2 * V], F32)
                nc.tensor.matmul(d2t[:], lhsT=pxf[:], rhs=rhs8[:],
                                 start=True, stop=True)

                d2c = pool.tile([P, V], F32)
                dist = pool.tile([P, V], F32)
                pen = pool.tile([P, V], F32)
                de = pool.tile([P, V], F32)
                E = pool.tile([P, V], F32)
                Wt = pool.tile([P, V], F32)
                omw = pool.tile([P, V], F32)
                Pbuf = pool.tile([P, V + 1], F32)
                omb = pool.tile([P, V], F32)
                Qbuf = pool.tile([P, V + 1], F32)
                BQ = pool.tile([P, V], F32)

                nc.scalar.activation(out=d2c[:], in_=d2t[:, 0:V], func=Act.Relu)
                nc.scalar.activation(out=dist[:], in_=d2c[:], func=Act.Sqrt)
                nc.vector.tensor_scalar(out=pen[:], in0=d2t[:, V:2 * V],
                                        scalar1=BIG, scalar2=0.0,
                                        op0=Alu.mult, op1=Alu.max)
                nc.gpsimd.tensor_tensor(out=de[:], in0=dist[:], in1=pen[:],
                                        op=Alu.add)
                nc.scalar.activation(out=E[:], in_=de[:], func=Act.Exp,
                                     scale=-0.5)
                nc.gpsimd.tensor_tensor(out=Wt[:], in0=E[:], in1=vab[:],
                                        op=Alu.mult)
                nc.vector.tensor_scalar(out=omw[:], in0=Wt[:], scalar1=-1.0,
                                        scalar2=1.0, op0=Alu.mult, op1=Alu.add)
                nc.gpsimd.memset(Pbuf[:, 0:1], 1.0)
                nc.gpsimd.memset(Qbuf[:, V:V + 1], 1.0)
                nc.vector.tensor_tensor_scan(
                    out=Pbuf[:, 1:V + 1], data0=omw[:], data1=zeros[:],
                    initial=1.0, op0=Alu.mult, op1=Alu.max)
                nc.vector.scalar_tensor_tensor(
                    out=omb[:], in0=Pbuf[:, 1:V + 1], scalar=1.0,
                    in1=Pbuf[:, 0:V], op0=Alu.add, op1=Alu.subtract)
                nc.vector.tensor_tensor_scan(
                    out=Qbuf[:, 0:V][:, ::-1], data0=omb[:][:, ::-1],
                    data1=zeros[:], initial=1.0, op0=Alu.mult, op1=Alu.max)
                nc.vector.tensor_tensor(out=BQ[:], in0=Qbuf[:, 1:V + 1],
                                        in1=Qbuf[:, 0:V], op=Alu.subtract)
                nc.gpsimd.tensor_copy(out=astage[:, t:t + 1],
                                      in_=Pbuf[:, V:V + 1])

                bqT = ppool.tile([P, VC * P], F32)
                bqTs = pool.tile([P, VC * P], F32)
                for ch in range(VC):
                    cl = min(P, V - ch * P)
                    nc.tensor.transpose(bqT[0:cl, ch * P:ch * P + P],
                                        BQ[:, ch * P:ch * P + cl], ident[:])
                    nc.scalar.copy(out=bqTs[0:cl, ch * P:ch * P + P],
                                   in_=bqT[0:cl, ch * P:ch * P + P])
                for ch in range(VC):
                    cl = min(P, V - ch * P)
                    nc.tensor.matmul(outp[:, 4 * t:4 * t + 4],
                                     lhsT=bqTs[0:cl, ch * P:ch * P + P],
                                     rhs=vals[0:cl, ch, :],
                                     start=(ch == 0), stop=(ch == VC - 1))

            rgbsb = cpool.tile([P, T, 3], F32)
            depsb = cpool.tile([P, T], F32)
            alpsb = cpool.tile([P, T], F32)
            outv = outp[:].rearrange("p (t f) -> p t f", f=4)
            nc.vector.tensor_copy(out=rgbsb[:], in_=outv[:, :, 0:3])
            nc.vector.tensor_scalar(out=depsb[:], in0=outv[:, :, 3:4],
                                    scalar1=1.0, scalar2=FAR,
                                    op0=Alu.mult, op1=Alu.add)
            nc.vector.tensor_scalar(out=alpsb[:], in0=astage[:], scalar1=-1.0,
                                    scalar2=1.0, op0=Alu.mult, op1=Alu.add)
            nc.sync.dma_start(out=rgb_d.ap(), in_=rgbsb[:])
            nc.sync.dma_start(out=dep_d.ap(), in_=depsb[:])
            nc.sync.dma_start(out=alp_d.ap(), in_=alpsb[:])
    nc.compile()
    return nc


def _softplus(x):
    return np.logaddexp(0.0, x)


def _prep(positions, sizes, densities, colors, camera_matrix, intrinsics):
    """Mirror the reference projection in numpy fp32; build per-tile tables."""
    f32 = np.float32
    pos = positions.astype(f32)
    n = pos.shape[0]
    hom = np.concatenate([pos, np.ones((n, 1), f32)], axis=1)
    cam = hom @ camera_matrix.astype(f32).T
    with np.errstate(divide="ignore", invalid="ignore", over="ignore"):
        cam3 = cam[:, :3] / cam[:, 3:4]
        scr = cam3 @ intrinsics.astype(f32).T
        sp = scr[:, :2] / scr[:, 2:3]
        depths = cam3[:, 2]
        fx = intrinsics.astype(f32)[0, 0]
        ssize = sizes.astype(f32) * fx / np.maximum(depths, f32(0.1))
        x, y = sp[:, 0], sp[:, 1]
        vis = ((depths > NEAR) & (depths < FAR)
               & (x + ssize >= 0) & (x - ssize < W_IMG)
               & (y + ssize >= 0) & (y - ssize < H))
        vis = vis & np.isfinite(x) & np.isfinite(y) & np.isfinite(ssize)
        valpha = np.clip(
            1.0 - np.exp(-_softplus(densities.astype(f32)) * sizes.astype(f32)),
            0.0, 1.0) * vis.astype(f32)
    order = np.argsort(-np.where(np.isnan(depths), -np.inf, depths),
                       kind="stable")
    sp_s = sp[order].astype(np.float64)
    hs_s = 0.5 * ssize[order].astype(np.float64)
    va_s = valpha[order].astype(np.float64)
    d_s = depths[order].astype(np.float64)
    c_s = colors.astype(f32)[order].astype(np.float64)

    live = (va_s > 0) & (hs_s >= 0) & np.isfinite(hs_s) \
        & np.isfinite(sp_s).all(axis=1)
    sx = np.clip(sp_s[:, 0], -1e15, 1e15)
    sy = np.clip(sp_s[:, 1], -1e15, 1e15)
    hs2 = np.minimum(hs_s * hs_s, 1e12)

    # per (core, tile) voxel index lists (order preserved = back-to-front)
    lists = []
    vmax = 1
    m = 0.1
    for k in range(NCORES):
        row0 = k * BAND
        core_lists = []
        for tr in range(TR):
            for tcc in range(TC):
                y0, x0 = row0 + tr * R_, tcc * C_
                sel = (live
                       & (sx + hs_s + m >= x0) & (sx - hs_s - m <= x0 + C_ - 1)
                       & (sy + hs_s + m >= y0) & (sy - hs_s - m <= y0 + R_ - 1))
                idx = np.nonzero(sel)[0]
                core_lists.append(idx)
                vmax = max(vmax, len(idx))
        lists.append(core_lists)

    if vmax <= 128:
        V = max(16, (vmax + 15) // 16 * 16)
    else:
        V = (vmax + P - 1) // P * P
    VC = (V + P - 1) // P

    in_maps = []
    for k in range(NCORES):
        pxf = np.zeros((T, 4, P), np.float32)
        rhs8 = np.zeros((T, 4, 2 * V), np.float32)
        vab = np.zeros((T, V), np.float32)
        vals = np.zeros((T, V, 4), np.float32)
        row0 = k * BAND
        for t in range(T):
            tr, tcc = divmod(t, TC)
            y0, x0 = row0 + tr * R_, tcc * C_
            cx0, cy0 = x0 + (C_ - 1) / 2.0, y0 + (R_ - 1) / 2.0
            pxx = np.tile(np.arange(C_), R_) + x0 - cx0       # p = r*C_+c
            pyy = np.repeat(np.arange(R_), C_) + y0 - cy0
            pxf[t, 0], pxf[t, 1] = pxx, pyy
            pxf[t, 2] = pxx * pxx + pyy * pyy
            pxf[t, 3] = 1.0
            idx = lists[k][t]
            nv = len(idx)
            sxc, syc = sx[idx] - cx0, sy[idx] - cy0
            s2 = sxc * sxc + syc * syc
            rhs8[t, 0, 0:nv] = -2 * sxc
            rhs8[t, 1, 0:nv] = -2 * syc
            rhs8[t, 2, 0:nv] = 1.0
            rhs8[t, 3, 0:nv] = s2
            rhs8[t, 0, V:V + nv] = -2 * sxc
            rhs8[t, 1, V:V + nv] = -2 * syc
            rhs8[t, 2, V:V + nv] = 1.0
            rhs8[t, 3, V:V + nv] = s2 - hs2[idx]
            # padding voxels: rhs rows already 0 -> d2=0, t=0 -> masked only
            # by va=0; set pad mask row so t>0 kills them regardless
            if nv < V:
                rhs8[t, 2, nv:V] = 1.0
                rhs8[t, 3, nv:V] = 1e6          # d2 = 1e6 -> exp(-500) = 0
                rhs8[t, 2, V + nv:2 * V] = 1.0
                rhs8[t, 3, V + nv:2 * V] = 1e6  # t = 1e6 -> pen huge
            vab[t, 0:nv] = va_s[idx]
            vals[t, 0:nv, 0:3] = c_s[idx]
            vals[t, 0:nv, 3] = np.clip(d_s[idx], -1e30, 1e30) - FAR
        vals_pad = np.zeros((T, VC * P, 4), np.float32)
        vals_pad[:, 0:V, :] = vals
        vals_ch = vals_pad.reshape(T, VC, P, 4).transpose(0, 2, 1, 3).copy()
        in_maps.append({"pxf": pxf, "rhs8": rhs8, "vab": vab,
                        "vals": vals_ch,
                        "ident": np.eye(P, dtype=np.float32)})
    return V, in_maps


def kernel(positions, sizes, densities, colors, camera_matrix, intrinsics):
    from concourse import bass_utils

    V, in_maps = _prep(positions, sizes, densities, colors,
                       camera_matrix, intrinsics)
    if V not in _nc_cache:
        _nc_cache[V] = _build(V)
    nc = _nc_cache[V]
    res = bass_utils.run_bass_kernel_spmd(nc, in_maps,
                                          core_ids=list(range(NCORES)))
    rgb = np.zeros((H, W_IMG, 3), np.float32)
    dep = np.zeros((H, W_IMG), np.float32)
    alp = np.zeros((H, W_IMG), np.float32)
    # unscatter: pixel p = r*C_+c of tile t = tr*TC+tc -> (k*BAND+tr*R_+r,
    # tc*C_+c)
    for k in range(NCORES):
        r = res.results[k]
        band_rgb = (r["rgb"].reshape(R_, C_, TR, TC, 3)
                    .transpose(2, 0, 3, 1, 4).reshape(BAND, W_IMG, 3))
        band_dep = (r["dep"].reshape(R_, C_, TR, TC)
                    .transpose(2, 0, 3, 1).reshape(BAND, W_IMG))
        band_alp = (r["alp"].reshape(R_, C_, TR, TC)
                    .transpose(2, 0, 3, 1).reshape(BAND, W_IMG))
        rgb[k * BAND:(k + 1) * BAND] = band_rgb
        dep[k * BAND:(k + 1) * BAND] = band_dep
        alp[k * BAND:(k + 1) * BAND] = band_alp
    return rgb, dep, alp
